# revision 5
# baseline (speedup 1.0000x reference)
"""TRN2 Bass kernel for nn_BasicBlockInstanceNorm (gnn_message_passing).

Algorithm (per conv, twice):
    out[r] = sum_k ( sum_{edges (k,j): out_map[k,j]=r} x[in_map[k,j]] ) @ W[k]
    (aggregate-then-transform: per output row r and offset k, sum the source
    rows first -- the sum commutes with the per-k linear map)
then instance-norm (+residual on conv2) + leaky relu.

Sharding: 8 cores, one batch instance per core (batch_idx is sorted), so
instance-norm stats are core-local.  Each core owns its output rows and
gathers source rows from a replicated source table in its own HBM via
indirect DMA (128 rows / instruction -- the Pool/Q7 descriptor-generation
rate is the kernel bottleneck).  Slots (r, k) with multiple edges are
pre-summed into "aux rows" on device (multi-pass gather-with-accumulate
into compact SBUF blocks, then written to the source table) so the main
pass is exactly one gather per slot.  y1 is exchanged between convs with
an AllGather.  PE does S^T transposes + W_cat matmuls; DVE/ACT drain PSUM
and apply norm/leaky -- all hidden under the gather stream.
"""
import sys
import types

import numpy as np

sys.path.insert(0, "/opt/trn_rl_repo")

import concourse.bacc as bacc
import concourse.bass as bass
import concourse.mybir as mybir
import concourse.tile as tile
from concourse.bass_utils import run_bass_kernel_spmd
from concourse.masks import make_identity

N = 100000
C = 64
K = 27
B = 8
EPS = 1e-5
SLOPE = 0.01
TP = 128
T = 100            # output tiles per core
RPAD = T * TP      # 12800 padded rows per core
ACOLS = 64         # aux slots per chunk = 128*ACOLS
AUX_CHUNK = TP * ACOLS
DUMMY = np.int32(1 << 30)

TRACE = False
LAST_EXEC_NS = None

F32 = mybir.dt.float32
I32 = mybir.dt.int32


def _register_ntff_hook():
    import antenv
    if "antenv.axon_hooks" not in sys.modules:
        mod = types.ModuleType("antenv.axon_hooks")
        mod._hook = None
        def set_hook(h):
            mod._hook = h
        def get_hook():
            return mod._hook
        mod.set_axon_ntff_profile_hook = set_hook
        mod.get_axon_ntff_profile_hook = get_hook
        sys.modules["antenv.axon_hooks"] = mod
        antenv.axon_hooks = mod
    m = sys.modules["antenv.axon_hooks"]
    if m.get_axon_ntff_profile_hook() is None:
        from trn_agent_boot.trn_boot import _ntff_profile_via_ctypes
        m.set_axon_ntff_profile_hook(
            _ntff_profile_via_ctypes("/opt/axon/libaxon_pjrt.so")
        )


def _build_edge_structure(in_map, out_map, b0):
    """Per-core slot structure shared by both convs.

    Returns per-core dict with:
      single_slot, single_src   -- slots with exactly one edge (global src ids)
      aux_counts [A]            -- per aux slot edge count (desc-sorted)
      aux_slot [A]              -- slot id of each aux slot
      aux_srcs  [A, maxc]       -- global src ids per aux slot (padded -1)
    slot id = r*K + k  (r = core-local row).
    """
    src = in_map.ravel().astype(np.int64)          # edge order: k-major
    dst = out_map.ravel().astype(np.int64)
    k_arr = np.repeat(np.arange(K, dtype=np.int64), N)
    core = np.searchsorted(b0[1:-1], dst, side="right")
    r = dst - b0[core]
    gslot = (core * RPAD + r) * K + k_arr          # globally unique slot id
    order = np.argsort(gslot, kind="stable")
    gs = gslot[order]
    ss = src[order]
    uniq, start, cnt = np.unique(gs, return_index=True, return_counts=True)
    out = []
    for c in range(B):
        lo = np.searchsorted(uniq, c * RPAD * K)
        hi = np.searchsorted(uniq, (c + 1) * RPAD * K)
        u = uniq[lo:hi] - c * RPAD * K
        st = start[lo:hi]
        ct = cnt[lo:hi]
        singles = ct == 1
        single_slot = u[singles]
        single_src = ss[st[singles]]
        dmask = ct >= 2
        d_slot = u[dmask]
        d_start = st[dmask]
        d_cnt = ct[dmask]
        ordd = np.argsort(-d_cnt, kind="stable")
        d_slot = d_slot[ordd]
        d_start = d_start[ordd]
        d_cnt = d_cnt[ordd]
        maxc = int(d_cnt[0]) if len(d_cnt) else 0
        srcs = np.full((len(d_slot), maxc), -1, np.int64)
        for p in range(maxc):
            m = d_cnt > p
            srcs[m, p] = ss[d_start[m] + p]
        out.append(dict(single_slot=single_slot, single_src=single_src,
                        aux_cnt=d_cnt, aux_slot=d_slot, aux_srcs=srcs))
    return out


def _build_conv_arrays(structs, enc, zrow, auxbase):
    """Build per-core main idx [128, T*K] and aux pass arrays + global schedule.

    enc: vectorized map global-src -> xsrc row.  Returns (m_list, a_list,
    nchunks, npasses[chunk], ncols[chunk][pass], acalls).
    """
    maxA = max(len(s["aux_slot"]) for s in structs)
    nchunks = max(1, -(-maxA // AUX_CHUNK))
    apad = nchunks * AUX_CHUNK
    # global schedule
    npasses = []
    ncols = []
    for q in range(nchunks):
        pmax = 1
        for s in structs:
            cnts = s["aux_cnt"][q * AUX_CHUNK:(q + 1) * AUX_CHUNK]
            if len(cnts):
                pmax = max(pmax, int(cnts[0]))
        npasses.append(pmax)
        cols_q = []
        for p in range(pmax):
            if p == 0:
                cols_q.append(ACOLS)
                continue
            w = 1
            for s in structs:
                cnts = s["aux_cnt"][q * AUX_CHUNK:(q + 1) * AUX_CHUNK]
                pref = int((cnts > p).sum())
                w = max(w, -(-pref // TP)) if pref else w
            cols_q.append(w)
        ncols.append(cols_q)
    acalls = sum(sum(cq) for cq in ncols)

    m_list, a_list = [], []
    for s in structs:
        m = np.full((TP, T * K), zrow, np.int32)
        sl = s["single_slot"]
        rr = sl // K
        kk = sl % K
        tt = rr // TP
        pp = rr % TP
        m[pp, tt * K + kk] = enc(s["single_src"])
        # aux slots: main idx points at the aux row
        A = len(s["aux_slot"])
        a_ids = np.arange(apad, dtype=np.int64)
        chunk = a_ids // AUX_CHUNK
        w = a_ids % AUX_CHUNK
        dram_row = chunk * AUX_CHUNK + (w % TP) * ACOLS + (w // TP)
        if A:
            sl = s["aux_slot"]
            rr = sl // K
            kk = sl % K
            tt = rr // TP
            pp = rr % TP
            m[pp, tt * K + kk] = (auxbase + dram_row[:A]).astype(np.int32)
        a = np.full((TP, acalls), DUMMY, np.int32)
        col = 0
        for q in range(nchunks):
            base = q * AUX_CHUNK
            for p in range(npasses[q]):
                for cq in range(ncols[q][p]):
                    ws = base + cq * TP + np.arange(TP)
                    vals = np.full(TP, DUMMY, np.int32)
                    if p == 0:
                        vals[:] = zrow
                    inr = ws < A
                    if inr.any():
                        wi = ws[inr]
                        have = s["aux_srcs"].shape[1] > p
                        if have:
                            sv = s["aux_srcs"][wi, p]
                            ok = sv >= 0
                            tmp = vals[inr]
                            tmp[ok] = enc(sv[ok])
                            vals[inr] = tmp
                    a[:, col] = vals
                    col += 1
        m_list.append(m)
        a_list.append(a)
    return m_list, a_list, nchunks, npasses, ncols, acalls, apad


def _emit_conv(nc, tc, pools, cfg):
    """Emit one conv's instructions."""
    (const_p, s_pool, st_pool, aux_pool, small, psumT, psumZ, psumY) = pools
    midx = cfg["midx"]
    aidx = cfg["aidx"]
    xsrc = cfg["xsrc"]
    wslab = cfg["wslab"]
    ident = cfg["ident"]
    nsrc = cfg["nsrc"]
    auxbase = cfg["auxbase"]
    nchunks, npasses, ncols = cfg["sched"]
    zslab = cfg["zslab"]
    sums = cfg["sums"]
    sqs = cfg["sqs"]

    AX = mybir.AxisListType.X
    # ---- aux build ----
    acall = 0
    for q in range(nchunks):
        slab = aux_pool.tile([TP, ACOLS * C], F32, tag="aux")
        for p in range(npasses[q]):
            for cq in range(ncols[q][p]):
                dst = slab[:, cq * C:(cq + 1) * C] if p > 0 else None
                col = aidx[:, acall:acall + 1]
                acall += 1
                if p == 0:
                    nc.gpsimd.indirect_dma_start(
                        out=slab[:, cq * C:(cq + 1) * C], out_offset=None,
                        in_=xsrc.ap(),
                        in_offset=bass.IndirectOffsetOnAxis(ap=col, axis=0))
                else:
                    nc.gpsimd.indirect_dma_start(
                        out=dst, out_offset=None, in_=xsrc.ap(),
                        in_offset=bass.IndirectOffsetOnAxis(ap=col, axis=0),
                        bounds_check=nsrc - 1, oob_is_err=False,
                        compute_op=mybir.AluOpType.add)
        row0 = auxbase + q * AUX_CHUNK
        nc.sync.dma_start(
            out=xsrc.ap()[row0:row0 + AUX_CHUNK].rearrange(
                "(p a) c -> p (a c)", p=TP),
            in_=slab[:])

    # ---- main pass + matmul + stats ----
    for t in range(T):
        S = s_pool.tile([TP, 14 * TP], F32, tag="S")
        nc.vector.memset(S[:, K * C:], 0.0)
        for k in range(K):
            nc.gpsimd.indirect_dma_start(
                out=S[:, k * C:(k + 1) * C], out_offset=None, in_=xsrc.ap(),
                in_offset=bass.IndirectOffsetOnAxis(
                    ap=midx[:, t * K + k:t * K + k + 1], axis=0))
        zT = psumZ.tile([C, TP], F32, tag="zT")
        for i in range(14):
            tp_ = psumT.tile([TP, TP], F32, tag="tp")
            nc.tensor.transpose(tp_[:], S[:, i * TP:(i + 1) * TP], ident[:])
            stc = st_pool.tile([TP, TP], F32, tag="st")
            if i % 2 == 0:
                nc.vector.tensor_copy(out=stc[:], in_=tp_[:])
            else:
                nc.scalar.copy(out=stc[:], in_=tp_[:])
            nc.tensor.matmul(zT[:], lhsT=wslab[:, i * C:(i + 1) * C],
                             rhs=stc[:], start=(i == 0), stop=(i == 13))
        nc.vector.reduce_sum(out=sums[:, t:t + 1], in_=zT[:], axis=AX)
        sq = small.tile([C, TP], F32, tag="sq")
        nc.scalar.activation(out=sq[:], in_=zT[:],
                             func=mybir.ActivationFunctionType.Square,
                             accum_out=sqs[:, t:t + 1])
        nc.vector.tensor_copy(out=zslab[:, t * TP:(t + 1) * TP], in_=zT[:])


def _emit_norm_epilogue(nc, tc, pools, cfg):
    (const_p, s_pool, st_pool, aux_pool, small, psumT, psumZ, psumY) = pools
    zslab = cfg["zslab"]
    sums = cfg["sums"]
    sqs = cfg["sqs"]
    invct = cfg["invct"]
    epsT = cfg["epsT"]
    g_t = cfg["g_t"]
    b_t = cfg["b_t"]
    ident = cfg["ident"]
    AX = mybir.AxisListType.X
    ALU = mybir.AluOpType

    sumv = small.tile([C, 1], F32, tag="n1")
    nc.vector.reduce_sum(out=sumv[:], in_=sums[:], axis=AX)
    sqv = small.tile([C, 1], F32, tag="n2")
    nc.vector.reduce_sum(out=sqv[:], in_=sqs[:], axis=AX)
    mean = small.tile([C, 1], F32, tag="n3")
    nc.vector.tensor_mul(out=mean[:], in0=sumv[:], in1=invct[:])
    msq = small.tile([C, 1], F32, tag="n4")
    nc.vector.tensor_mul(out=msq[:], in0=sqv[:], in1=invct[:])
    m2t = small.tile([C, 1], F32, tag="n5")
    nc.vector.tensor_mul(out=m2t[:], in0=mean[:], in1=mean[:])
    var = small.tile([C, 1], F32, tag="n6")
    nc.vector.tensor_tensor(out=var[:], in0=msq[:], in1=m2t[:],
                            op=ALU.subtract)
    stdt = small.tile([C, 1], F32, tag="n7")
    nc.scalar.activation(out=stdt[:], in_=var[:],
                         func=mybir.ActivationFunctionType.Sqrt,
                         bias=epsT[:], scale=1.0)
    rstd = small.tile([C, 1], F32, tag="n8")
    nc.vector.reciprocal(out=rstd[:], in_=stdt[:])
    scl = small.tile([C, 1], F32, tag="n9")
    nc.vector.tensor_mul(out=scl[:], in0=g_t[:], in1=rstd[:])
    tmpb = small.tile([C, 1], F32, tag="n10")
    nc.vector.tensor_mul(out=tmpb[:], in0=mean[:], in1=scl[:])
    nbias = small.tile([C, 1], F32, tag="n11")
    nc.vector.tensor_tensor(out=nbias[:], in0=b_t[:], in1=tmpb[:],
                            op=ALU.subtract)
    return scl, nbias


def kernel(feats, W1, g1, b1, W2, g2, b2, in_map, out_map, batch_idx):
    global LAST_EXEC_NS
    feats = np.ascontiguousarray(np.asarray(feats, np.float32))
    in_map = np.asarray(in_map, np.int32)
    out_map = np.asarray(out_map, np.int32)
    batch_idx = np.asarray(batch_idx, np.int32)

    counts = np.bincount(batch_idx, minlength=B)
    b0 = np.zeros(B + 1, np.int64)
    b0[1:] = np.cumsum(counts)
    assert b0[-1] == N

    structs = _build_edge_structure(in_map, out_map, b0)

    # conv1 source table: [feats | zero | aux1]
    Z1 = N
    AB1 = N + 1
    m1, a1, nch1, npass1, ncols1, acalls1, apad1 = _build_conv_arrays(
        structs, lambda s: np.asarray(s, np.int64).astype(np.int32), Z1, AB1)
    # conv2 source table: [y1 padded blocks | zero | aux2]
    Z2 = B * RPAD
    AB2 = B * RPAD + 1
    src_core = np.searchsorted(b0[1:-1], np.arange(N), side="right")
    enc2_tab = (src_core * RPAD + (np.arange(N) - b0[src_core])).astype(np.int32)
    m2, a2, nch2, npass2, ncols2, acalls2, apad2 = _build_conv_arrays(
        structs, lambda s: enc2_tab[np.asarray(s, np.int64)], Z2, AB2)

    w1cat = np.zeros((14 * TP, C), np.float32)
    w1cat[:K * C] = np.asarray(W1, np.float32).reshape(K * C, C)
    w2cat = np.zeros((14 * TP, C), np.float32)
    w2cat[:K * C] = np.asarray(W2, np.float32).reshape(K * C, C)

    fpads = []
    for c in range(B):
        fp = np.zeros((RPAD, C), np.float32)
        fp[:counts[c]] = feats[b0[c]:b0[c + 1]]
        fpads.append(fp)

    nsrc1 = N + 1 + apad1
    nsrc2 = B * RPAD + 1 + apad2

    # ---------------- build program ----------------
    nc = bacc.Bacc("TRN2", target_bir_lowering=False, debug=False,
                   num_devices=B)
    feats_t = nc.dram_tensor("feats", [N, C], F32, kind="ExternalInput")
    fpad_t = nc.dram_tensor("fpad", [RPAD, C], F32, kind="ExternalInput")
    w1_t = nc.dram_tensor("w1cat", [14 * TP, C], F32, kind="ExternalInput")
    w2_t = nc.dram_tensor("w2cat", [14 * TP, C], F32, kind="ExternalInput")
    gb_t = nc.dram_tensor("gb", [4, C], F32, kind="ExternalInput")
    inv_t = nc.dram_tensor("invc", [1, 1], F32, kind="ExternalInput")
    m1_t = nc.dram_tensor("m1", [TP, T * K], I32, kind="ExternalInput")
    m2_t = nc.dram_tensor("m2", [TP, T * K], I32, kind="ExternalInput")
    a1_t = nc.dram_tensor("a1", [TP, max(acalls1, 1)], I32, kind="ExternalInput")
    a2_t = nc.dram_tensor("a2", [TP, max(acalls2, 1)], I32, kind="ExternalInput")
    out_t = nc.dram_tensor("out", [RPAD, C], F32, kind="ExternalOutput")
    xsrc1 = nc.dram_tensor("xsrc1", [nsrc1, C], F32)
    xsrc2 = nc.dram_tensor("xsrc2", [nsrc2, C], F32)
    y1loc = nc.dram_tensor("y1loc", [RPAD, C], F32)

    with tile.TileContext(nc) as tc:
        with (
            tc.tile_pool(name="const", bufs=1) as const_p,
            tc.tile_pool(name="S", bufs=3) as s_pool,
            tc.tile_pool(name="st", bufs=4) as st_pool,
            tc.tile_pool(name="aux", bufs=2) as aux_pool,
            tc.tile_pool(name="small", bufs=3) as small,
            tc.tile_pool(name="zsl", bufs=1) as zpool,
            tc.tile_pool(name="psT", bufs=3, space="PSUM") as psumT,
            tc.tile_pool(name="psZ", bufs=2, space="PSUM") as psumZ,
            tc.tile_pool(name="psY", bufs=2, space="PSUM") as psumY,
        ):
            pools = (const_p, s_pool, st_pool, aux_pool, small, psumT,
                     psumZ, psumY)
            ident = const_p.tile([TP, TP], F32)
            make_identity(nc, ident[:])
            w1s = const_p.tile([TP, 14 * C], F32)
            nc.sync.dma_start(
                out=w1s[:].rearrange("p (i c) -> p i c", c=C),
                in_=w1_t.ap().rearrange("(i p) c -> p i c", p=TP))
            w2s = const_p.tile([TP, 14 * C], F32)
            nc.sync.dma_start(
                out=w2s[:].rearrange("p (i c) -> p i c", c=C),
                in_=w2_t.ap().rearrange("(i p) c -> p i c", p=TP))
            gvec = []
            for i in range(4):
                gt = const_p.tile([C, 1], F32, tag=f"gb{i}")
                nc.sync.dma_start(out=gt[:], in_=gb_t.ap()[i:i+1].rearrange("a c -> c a"))
                gvec.append(gt)
            invct = const_p.tile([C, 1], F32)
            nc.sync.dma_start(
                out=invct[:],
                in_=bass.AP(tensor=inv_t, offset=0, ap=[[0, C], [1, 1]]))
            epsT = const_p.tile([C, 1], F32)
            nc.vector.memset(epsT[:], EPS)
            zrow = const_p.tile([1, C], F32)
            nc.vector.memset(zrow[:], 0.0)

            m1s = const_p.tile([TP, T * K], I32)
            nc.sync.dma_start(out=m1s[:], in_=m1_t.ap())
            m2s = const_p.tile([TP, T * K], I32)
            nc.sync.dma_start(out=m2s[:], in_=m2_t.ap())
            a1s = const_p.tile([TP, max(acalls1, 1)], I32)
            nc.sync.dma_start(out=a1s[:], in_=a1_t.ap())
            a2s = const_p.tile([TP, max(acalls2, 1)], I32)
            nc.sync.dma_start(out=a2s[:], in_=a2_t.ap())

            zslab = zpool.tile([C, RPAD], F32)
            sums1 = const_p.tile([C, T], F32, tag="sums1")
            sqs1 = const_p.tile([C, T], F32, tag="sqs1")
            sums2 = const_p.tile([C, T], F32, tag="sums2")
            sqs2 = const_p.tile([C, T], F32, tag="sqs2")

            # init conv1 source table
            nc.sync.dma_start(out=xsrc1.ap()[0:N], in_=feats_t.ap())
            nc.sync.dma_start(out=xsrc1.ap()[N:N + 1], in_=zrow[:])

            cfg1 = dict(midx=m1s, aidx=a1s, xsrc=xsrc1, wslab=w1s,
                        ident=ident, nsrc=nsrc1, auxbase=AB1,
                        sched=(nch1, npass1, ncols1), zslab=zslab,
                        sums=sums1, sqs=sqs1, invct=invct, epsT=epsT,
                        g_t=gvec[0], b_t=gvec[1])
            _emit_conv(nc, tc, pools, cfg1)
            scl1, nb1 = _emit_norm_epilogue(nc, tc, pools, cfg1)

            ALU = mybir.AluOpType
            for t in range(T):
                yT = small.tile([C, TP], F32, tag="yT")
                nc.vector.tensor_scalar(
                    out=yT[:], in0=zslab[:, t * TP:(t + 1) * TP],
                    scalar1=scl1[:], scalar2=nb1[:],
                    op0=ALU.mult, op1=ALU.add)
                tmp = small.tile([C, TP], F32, tag="yt2")
                nc.scalar.mul(tmp[:], yT[:], SLOPE)
                nc.vector.tensor_tensor(out=yT[:], in0=yT[:], in1=tmp[:],
                                        op=ALU.max)
                ytp = psumY.tile([TP, C], F32, tag="ytp")
                nc.tensor.transpose(ytp[:], yT[:], ident[:C, :C])
                yrow = small.tile([TP, C], F32, tag="yrow")
                nc.scalar.copy(out=yrow[:], in_=ytp[:])
                nc.sync.dma_start(out=y1loc.ap()[t * TP:(t + 1) * TP],
                                  in_=yrow[:])

            # exchange y1
            nc.gpsimd.collective_compute(
                "AllGather", mybir.AluOpType.bypass,
                replica_groups=[list(range(B))],
                ins=[y1loc.ap()],
                outs=[xsrc2.ap()[0:B * RPAD]],
            )
            nc.sync.dma_start(out=xsrc2.ap()[Z2:Z2 + 1], in_=zrow[:])

            cfg2 = dict(midx=m2s, aidx=a2s, xsrc=xsrc2, wslab=w2s,
                        ident=ident, nsrc=nsrc2, auxbase=AB2,
                        sched=(nch2, npass2, ncols2), zslab=zslab,
                        sums=sums2, sqs=sqs2, invct=invct, epsT=epsT,
                        g_t=gvec[2], b_t=gvec[3])
            _emit_conv(nc, tc, pools, cfg2)
            scl2, nb2 = _emit_norm_epilogue(nc, tc, pools, cfg2)

            for t in range(T):
                nT = small.tile([C, TP], F32, tag="nT")
                nc.vector.tensor_scalar(
                    out=nT[:], in0=zslab[:, t * TP:(t + 1) * TP],
                    scalar1=scl2[:], scalar2=nb2[:],
                    op0=ALU.mult, op1=ALU.add)
                ntp = psumY.tile([TP, C], F32, tag="ytp")
                nc.tensor.transpose(ntp[:], nT[:], ident[:C, :C])
                ft = small.tile([TP, C], F32, tag="ft")
                nc.sync.dma_start(out=ft[:],
                                  in_=fpad_t.ap()[t * TP:(t + 1) * TP])
                radd = small.tile([TP, C], F32, tag="radd")
                nc.vector.tensor_add(out=radd[:], in0=ntp[:], in1=ft[:])
                tmp2 = small.tile([TP, C], F32, tag="tmp2")
                nc.scalar.mul(tmp2[:], radd[:], SLOPE)
                orow = small.tile([TP, C], F32, tag="orow")
                nc.vector.tensor_tensor(out=orow[:], in0=radd[:], in1=tmp2[:],
                                        op=ALU.max)
                nc.sync.dma_start(out=out_t.ap()[t * TP:(t + 1) * TP],
                                  in_=orow[:])

    nc.compile()

    in_maps = []
    for c in range(B):
        gb = np.stack([np.asarray(g1, np.float32), np.asarray(b1, np.float32),
                       np.asarray(g2, np.float32), np.asarray(b2, np.float32)])
        in_maps.append({
            "feats": feats,
            "fpad": fpads[c],
            "w1cat": w1cat,
            "w2cat": w2cat,
            "gb": gb,
            "invc": np.array([[1.0 / max(int(counts[c]), 1)]], np.float32),
            "m1": m1[c], "m2": m2[c],
            "a1": a1[c] if acalls1 else np.zeros((TP, 1), np.int32),
            "a2": a2[c] if acalls2 else np.zeros((TP, 1), np.int32),
        })

    if TRACE:
        _register_ntff_hook()
    br = run_bass_kernel_spmd(nc, in_maps, list(range(B)), trace=TRACE)
    LAST_EXEC_NS = br.exec_time_ns

    out = np.empty((N, C), np.float32)
    for c in range(B):
        out[b0[c]:b0[c + 1]] = br.results[c]["out"][:counts[c]]
    return out


# revision 6
# speedup vs baseline: 1.0060x; 1.0060x over previous
"""TRN2 Bass kernel for nn_BasicBlockInstanceNorm (gnn_message_passing).

Algorithm (per conv, twice):
    out[r] = sum_k ( sum_{edges (k,j): out_map[k,j]=r} x[in_map[k,j]] ) @ W[k]
    (aggregate-then-transform: per output row r and offset k, sum the source
    rows first -- the sum commutes with the per-k linear map)
then instance-norm (+residual on conv2) + leaky relu.

Sharding: 8 cores, one batch instance per core (batch_idx is sorted), so
instance-norm stats are core-local.  Each core owns its output rows and
gathers source rows from a replicated source table in its own HBM via
indirect DMA (128 rows / instruction -- the Pool/Q7 descriptor-generation
rate is the kernel bottleneck).  Slots (r, k) with multiple edges are
pre-summed into "aux rows" on device (multi-pass gather-with-accumulate
into compact SBUF blocks, then written to the source table) so the main
pass is exactly one gather per slot.  y1 is exchanged between convs with
an AllGather.  PE does S^T transposes + W_cat matmuls; DVE/ACT drain PSUM
and apply norm/leaky -- all hidden under the gather stream.
"""
import sys
import types

import numpy as np

sys.path.insert(0, "/opt/trn_rl_repo")

import concourse.bacc as bacc
import concourse.bass as bass
import concourse.mybir as mybir
import concourse.tile as tile
from concourse.bass_utils import run_bass_kernel_spmd
from concourse.masks import make_identity

N = 100000
C = 64
K = 27
B = 8
EPS = 1e-5
SLOPE = 0.01
TP = 128
T = 100            # output tiles per core (recomputed per inputs)
RPAD = T * TP      # padded rows per core (recomputed per inputs)
ACOLS = 64         # aux slots per chunk = 128*ACOLS
AUX_CHUNK = TP * ACOLS
DUMMY = np.int32(1 << 30)

TRACE = False
LAST_EXEC_NS = None

F32 = mybir.dt.float32
I32 = mybir.dt.int32


def _register_ntff_hook():
    import antenv
    if "antenv.axon_hooks" not in sys.modules:
        mod = types.ModuleType("antenv.axon_hooks")
        mod._hook = None
        def set_hook(h):
            mod._hook = h
        def get_hook():
            return mod._hook
        mod.set_axon_ntff_profile_hook = set_hook
        mod.get_axon_ntff_profile_hook = get_hook
        sys.modules["antenv.axon_hooks"] = mod
        antenv.axon_hooks = mod
    m = sys.modules["antenv.axon_hooks"]
    if m.get_axon_ntff_profile_hook() is None:
        from trn_agent_boot.trn_boot import _ntff_profile_via_ctypes
        m.set_axon_ntff_profile_hook(
            _ntff_profile_via_ctypes("/opt/axon/libaxon_pjrt.so")
        )


def _build_edge_structure(in_map, out_map, b0):
    """Per-core slot structure shared by both convs.

    Returns per-core dict with:
      single_slot, single_src   -- slots with exactly one edge (global src ids)
      aux_counts [A]            -- per aux slot edge count (desc-sorted)
      aux_slot [A]              -- slot id of each aux slot
      aux_srcs  [A, maxc]       -- global src ids per aux slot (padded -1)
    slot id = r*K + k  (r = core-local row).
    """
    src = in_map.ravel().astype(np.int64)          # edge order: k-major
    dst = out_map.ravel().astype(np.int64)
    k_arr = np.repeat(np.arange(K, dtype=np.int64), N)
    core = np.searchsorted(b0[1:-1], dst, side="right")
    r = dst - b0[core]
    gslot = (core * RPAD + r) * K + k_arr          # globally unique slot id
    order = np.argsort(gslot, kind="stable")
    gs = gslot[order]
    ss = src[order]
    uniq, start, cnt = np.unique(gs, return_index=True, return_counts=True)
    out = []
    for c in range(B):
        lo = np.searchsorted(uniq, c * RPAD * K)
        hi = np.searchsorted(uniq, (c + 1) * RPAD * K)
        u = uniq[lo:hi] - c * RPAD * K
        st = start[lo:hi]
        ct = cnt[lo:hi]
        singles = ct == 1
        single_slot = u[singles]
        single_src = ss[st[singles]]
        dmask = ct >= 2
        d_slot = u[dmask]
        d_start = st[dmask]
        d_cnt = ct[dmask]
        ordd = np.argsort(-d_cnt, kind="stable")
        d_slot = d_slot[ordd]
        d_start = d_start[ordd]
        d_cnt = d_cnt[ordd]
        maxc = int(d_cnt[0]) if len(d_cnt) else 0
        srcs = np.full((len(d_slot), maxc), -1, np.int64)
        for p in range(maxc):
            m = d_cnt > p
            srcs[m, p] = ss[d_start[m] + p]
        out.append(dict(single_slot=single_slot, single_src=single_src,
                        aux_cnt=d_cnt, aux_slot=d_slot, aux_srcs=srcs))
    return out


def _build_conv_arrays(structs, enc, zrow, auxbase):
    """Build per-core main idx [128, T*K] and aux pass arrays + global schedule.

    enc: vectorized map global-src -> xsrc row.  Returns (m_list, a_list,
    nchunks, npasses[chunk], ncols[chunk][pass], acalls).
    """
    maxA = max(len(s["aux_slot"]) for s in structs)
    nchunks = max(1, -(-maxA // AUX_CHUNK))
    apad = nchunks * AUX_CHUNK
    # global schedule
    npasses = []
    ncols = []
    for q in range(nchunks):
        pmax = 1
        for s in structs:
            cnts = s["aux_cnt"][q * AUX_CHUNK:(q + 1) * AUX_CHUNK]
            if len(cnts):
                pmax = max(pmax, int(cnts[0]))
        npasses.append(pmax)
        cols_q = []
        for p in range(pmax):
            if p == 0:
                cols_q.append(ACOLS)
                continue
            w = 1
            for s in structs:
                cnts = s["aux_cnt"][q * AUX_CHUNK:(q + 1) * AUX_CHUNK]
                pref = int((cnts > p).sum())
                w = max(w, -(-pref // TP)) if pref else w
            cols_q.append(w)
        ncols.append(cols_q)
    acalls = sum(sum(cq) for cq in ncols)

    m_list, a_list = [], []
    for s in structs:
        m = np.full((TP, T * K), zrow, np.int32)
        sl = s["single_slot"]
        rr = sl // K
        kk = sl % K
        tt = rr // TP
        pp = rr % TP
        m[pp, tt * K + kk] = enc(s["single_src"])
        # aux slots: main idx points at the aux row
        A = len(s["aux_slot"])
        a_ids = np.arange(apad, dtype=np.int64)
        chunk = a_ids // AUX_CHUNK
        w = a_ids % AUX_CHUNK
        dram_row = chunk * AUX_CHUNK + (w % TP) * ACOLS + (w // TP)
        if A:
            sl = s["aux_slot"]
            rr = sl // K
            kk = sl % K
            tt = rr // TP
            pp = rr % TP
            m[pp, tt * K + kk] = (auxbase + dram_row[:A]).astype(np.int32)
        a = np.full((TP, acalls), DUMMY, np.int32)
        col = 0
        for q in range(nchunks):
            base = q * AUX_CHUNK
            for p in range(npasses[q]):
                for cq in range(ncols[q][p]):
                    ws = base + cq * TP + np.arange(TP)
                    vals = np.full(TP, DUMMY, np.int32)
                    if p == 0:
                        vals[:] = zrow
                    inr = ws < A
                    if inr.any():
                        wi = ws[inr]
                        have = s["aux_srcs"].shape[1] > p
                        if have:
                            sv = s["aux_srcs"][wi, p]
                            ok = sv >= 0
                            tmp = vals[inr]
                            tmp[ok] = enc(sv[ok])
                            vals[inr] = tmp
                    a[:, col] = vals
                    col += 1
        m_list.append(m)
        a_list.append(a)
    return m_list, a_list, nchunks, npasses, ncols, acalls, apad


def _emit_conv(nc, tc, pools, cfg):
    """Emit one conv's instructions."""
    (const_p, s_pool, st_pool, aux_pool, small, psumT, psumZ, psumY) = pools
    midx = cfg["midx"]
    aidx = cfg["aidx"]
    xsrc = cfg["xsrc"]
    wslab = cfg["wslab"]
    ident = cfg["ident"]
    nsrc = cfg["nsrc"]
    auxbase = cfg["auxbase"]
    nchunks, npasses, ncols = cfg["sched"]
    zslab = cfg["zslab"]
    sums = cfg["sums"]
    sqs = cfg["sqs"]

    AX = mybir.AxisListType.X
    # ---- aux build ----
    acall = 0
    for q in range(nchunks):
        slab = aux_pool.tile([TP, ACOLS * C], F32, tag="aux")
        for p in range(npasses[q]):
            for cq in range(ncols[q][p]):
                dst = slab[:, cq * C:(cq + 1) * C] if p > 0 else None
                col = aidx[:, acall:acall + 1]
                acall += 1
                if p == 0:
                    nc.gpsimd.indirect_dma_start(
                        out=slab[:, cq * C:(cq + 1) * C], out_offset=None,
                        in_=xsrc.ap(),
                        in_offset=bass.IndirectOffsetOnAxis(ap=col, axis=0))
                else:
                    nc.gpsimd.indirect_dma_start(
                        out=dst, out_offset=None, in_=xsrc.ap(),
                        in_offset=bass.IndirectOffsetOnAxis(ap=col, axis=0),
                        bounds_check=nsrc - 1, oob_is_err=False,
                        compute_op=mybir.AluOpType.add)
        row0 = auxbase + q * AUX_CHUNK
        nc.sync.dma_start(
            out=xsrc.ap()[row0:row0 + AUX_CHUNK].rearrange(
                "(p a) c -> p (a c)", p=TP),
            in_=slab[:])

    # ---- main pass + matmul + stats ----
    for t in range(T):
        S = s_pool.tile([TP, 14 * TP], F32, tag="S")
        nc.vector.memset(S[:, K * C:], 0.0)
        for k in range(K):
            nc.gpsimd.indirect_dma_start(
                out=S[:, k * C:(k + 1) * C], out_offset=None, in_=xsrc.ap(),
                in_offset=bass.IndirectOffsetOnAxis(
                    ap=midx[:, t * K + k:t * K + k + 1], axis=0))
        zT = psumZ.tile([C, TP], F32, tag="zT")
        for i in range(14):
            tp_ = psumT.tile([TP, TP], F32, tag="tp")
            nc.tensor.transpose(tp_[:], S[:, i * TP:(i + 1) * TP], ident[:])
            stc = st_pool.tile([TP, TP], F32, tag="st")
            if i % 2 == 0:
                nc.vector.tensor_copy(out=stc[:], in_=tp_[:])
            else:
                nc.scalar.copy(out=stc[:], in_=tp_[:])
            nc.tensor.matmul(zT[:], lhsT=wslab[:, i * C:(i + 1) * C],
                             rhs=stc[:], start=(i == 0), stop=(i == 13))
        nc.vector.reduce_sum(out=sums[:, t:t + 1], in_=zT[:], axis=AX)
        sq = small.tile([C, TP], F32, tag="sq")
        nc.scalar.activation(out=sq[:], in_=zT[:],
                             func=mybir.ActivationFunctionType.Square,
                             accum_out=sqs[:, t:t + 1])
        nc.vector.tensor_copy(out=zslab[:, t * TP:(t + 1) * TP], in_=zT[:])


def _emit_norm_epilogue(nc, tc, pools, cfg):
    (const_p, s_pool, st_pool, aux_pool, small, psumT, psumZ, psumY) = pools
    zslab = cfg["zslab"]
    sums = cfg["sums"]
    sqs = cfg["sqs"]
    invct = cfg["invct"]
    epsT = cfg["epsT"]
    g_t = cfg["g_t"]
    b_t = cfg["b_t"]
    ident = cfg["ident"]
    AX = mybir.AxisListType.X
    ALU = mybir.AluOpType

    sumv = small.tile([C, 1], F32, tag="n1")
    nc.vector.reduce_sum(out=sumv[:], in_=sums[:], axis=AX)
    sqv = small.tile([C, 1], F32, tag="n2")
    nc.vector.reduce_sum(out=sqv[:], in_=sqs[:], axis=AX)
    mean = small.tile([C, 1], F32, tag="n3")
    nc.vector.tensor_mul(out=mean[:], in0=sumv[:], in1=invct[:])
    msq = small.tile([C, 1], F32, tag="n4")
    nc.vector.tensor_mul(out=msq[:], in0=sqv[:], in1=invct[:])
    m2t = small.tile([C, 1], F32, tag="n5")
    nc.vector.tensor_mul(out=m2t[:], in0=mean[:], in1=mean[:])
    var = small.tile([C, 1], F32, tag="n6")
    nc.vector.tensor_tensor(out=var[:], in0=msq[:], in1=m2t[:],
                            op=ALU.subtract)
    stdt = small.tile([C, 1], F32, tag="n7")
    nc.scalar.activation(out=stdt[:], in_=var[:],
                         func=mybir.ActivationFunctionType.Sqrt,
                         bias=epsT[:], scale=1.0)
    rstd = small.tile([C, 1], F32, tag="n8")
    nc.vector.reciprocal(out=rstd[:], in_=stdt[:])
    scl = small.tile([C, 1], F32, tag="n9")
    nc.vector.tensor_mul(out=scl[:], in0=g_t[:], in1=rstd[:])
    tmpb = small.tile([C, 1], F32, tag="n10")
    nc.vector.tensor_mul(out=tmpb[:], in0=mean[:], in1=scl[:])
    nbias = small.tile([C, 1], F32, tag="n11")
    nc.vector.tensor_tensor(out=nbias[:], in0=b_t[:], in1=tmpb[:],
                            op=ALU.subtract)
    return scl, nbias


def kernel(feats, W1, g1, b1, W2, g2, b2, in_map, out_map, batch_idx):
    global LAST_EXEC_NS
    feats = np.ascontiguousarray(np.asarray(feats, np.float32))
    in_map = np.asarray(in_map, np.int32)
    out_map = np.asarray(out_map, np.int32)
    batch_idx = np.asarray(batch_idx, np.int32)

    global T, RPAD
    counts = np.bincount(batch_idx, minlength=B)
    T = max(1, -(-int(counts.max()) // TP))
    RPAD = T * TP
    b0 = np.zeros(B + 1, np.int64)
    b0[1:] = np.cumsum(counts)
    assert b0[-1] == N

    structs = _build_edge_structure(in_map, out_map, b0)

    # conv1 source table: [feats | zero | aux1]
    Z1 = N
    AB1 = N + 1
    m1, a1, nch1, npass1, ncols1, acalls1, apad1 = _build_conv_arrays(
        structs, lambda s: np.asarray(s, np.int64).astype(np.int32), Z1, AB1)
    # conv2 source table: [y1 padded blocks | zero | aux2]
    Z2 = B * RPAD
    AB2 = B * RPAD + 1
    src_core = np.searchsorted(b0[1:-1], np.arange(N), side="right")
    enc2_tab = (src_core * RPAD + (np.arange(N) - b0[src_core])).astype(np.int32)
    m2, a2, nch2, npass2, ncols2, acalls2, apad2 = _build_conv_arrays(
        structs, lambda s: enc2_tab[np.asarray(s, np.int64)], Z2, AB2)

    w1cat = np.zeros((14 * TP, C), np.float32)
    w1cat[:K * C] = np.asarray(W1, np.float32).reshape(K * C, C)
    w2cat = np.zeros((14 * TP, C), np.float32)
    w2cat[:K * C] = np.asarray(W2, np.float32).reshape(K * C, C)

    fpads = []
    for c in range(B):
        fp = np.zeros((RPAD, C), np.float32)
        fp[:counts[c]] = feats[b0[c]:b0[c + 1]]
        fpads.append(fp)

    nsrc1 = N + 1 + apad1
    nsrc2 = B * RPAD + 1 + apad2

    # ---------------- build program ----------------
    nc = bacc.Bacc("TRN2", target_bir_lowering=False, debug=False,
                   num_devices=B)
    feats_t = nc.dram_tensor("feats", [N, C], F32, kind="ExternalInput")
    fpad_t = nc.dram_tensor("fpad", [RPAD, C], F32, kind="ExternalInput")
    w1_t = nc.dram_tensor("w1cat", [14 * TP, C], F32, kind="ExternalInput")
    w2_t = nc.dram_tensor("w2cat", [14 * TP, C], F32, kind="ExternalInput")
    gb_t = nc.dram_tensor("gb", [4, C], F32, kind="ExternalInput")
    inv_t = nc.dram_tensor("invc", [1, 1], F32, kind="ExternalInput")
    m1_t = nc.dram_tensor("m1", [TP, T * K], I32, kind="ExternalInput")
    m2_t = nc.dram_tensor("m2", [TP, T * K], I32, kind="ExternalInput")
    a1_t = nc.dram_tensor("a1", [TP, max(acalls1, 1)], I32, kind="ExternalInput")
    a2_t = nc.dram_tensor("a2", [TP, max(acalls2, 1)], I32, kind="ExternalInput")
    out_t = nc.dram_tensor("out", [RPAD, C], F32, kind="ExternalOutput")
    xsrc1 = nc.dram_tensor("xsrc1", [nsrc1, C], F32)
    xsrc2 = nc.dram_tensor("xsrc2", [nsrc2, C], F32)
    y1loc = nc.dram_tensor("y1loc", [RPAD, C], F32)

    with tile.TileContext(nc) as tc:
        with (
            tc.tile_pool(name="const", bufs=1) as const_p,
            tc.tile_pool(name="S", bufs=5) as s_pool,
            tc.tile_pool(name="st", bufs=6) as st_pool,
            tc.tile_pool(name="aux", bufs=3) as aux_pool,
            tc.tile_pool(name="small", bufs=4) as small,
            tc.tile_pool(name="zsl", bufs=1) as zpool,
            tc.tile_pool(name="psT", bufs=3, space="PSUM") as psumT,
            tc.tile_pool(name="psZ", bufs=2, space="PSUM") as psumZ,
            tc.tile_pool(name="psY", bufs=2, space="PSUM") as psumY,
        ):
            pools = (const_p, s_pool, st_pool, aux_pool, small, psumT,
                     psumZ, psumY)
            ident = const_p.tile([TP, TP], F32)
            make_identity(nc, ident[:])
            w1s = const_p.tile([TP, 14 * C], F32)
            nc.sync.dma_start(
                out=w1s[:].rearrange("p (i c) -> p i c", c=C),
                in_=w1_t.ap().rearrange("(i p) c -> p i c", p=TP))
            w2s = const_p.tile([TP, 14 * C], F32)
            nc.sync.dma_start(
                out=w2s[:].rearrange("p (i c) -> p i c", c=C),
                in_=w2_t.ap().rearrange("(i p) c -> p i c", p=TP))
            gvec = []
            for i in range(4):
                gt = const_p.tile([C, 1], F32, tag=f"gb{i}")
                nc.sync.dma_start(out=gt[:], in_=gb_t.ap()[i:i+1].rearrange("a c -> c a"))
                gvec.append(gt)
            invct = const_p.tile([C, 1], F32)
            nc.sync.dma_start(
                out=invct[:],
                in_=bass.AP(tensor=inv_t, offset=0, ap=[[0, C], [1, 1]]))
            epsT = const_p.tile([C, 1], F32)
            nc.vector.memset(epsT[:], EPS)
            zrow = const_p.tile([1, C], F32)
            nc.vector.memset(zrow[:], 0.0)

            m1s = const_p.tile([TP, T * K], I32)
            nc.sync.dma_start(out=m1s[:], in_=m1_t.ap())
            m2s = const_p.tile([TP, T * K], I32)
            nc.sync.dma_start(out=m2s[:], in_=m2_t.ap())
            a1s = const_p.tile([TP, max(acalls1, 1)], I32)
            nc.sync.dma_start(out=a1s[:], in_=a1_t.ap())
            a2s = const_p.tile([TP, max(acalls2, 1)], I32)
            nc.sync.dma_start(out=a2s[:], in_=a2_t.ap())

            zslab = zpool.tile([C, RPAD], F32)
            sums1 = const_p.tile([C, T], F32, tag="sums1")
            sqs1 = const_p.tile([C, T], F32, tag="sqs1")
            sums2 = const_p.tile([C, T], F32, tag="sums2")
            sqs2 = const_p.tile([C, T], F32, tag="sqs2")

            # init conv1 source table
            nc.sync.dma_start(out=xsrc1.ap()[0:N], in_=feats_t.ap())
            nc.sync.dma_start(out=xsrc1.ap()[N:N + 1], in_=zrow[:])

            cfg1 = dict(midx=m1s, aidx=a1s, xsrc=xsrc1, wslab=w1s,
                        ident=ident, nsrc=nsrc1, auxbase=AB1,
                        sched=(nch1, npass1, ncols1), zslab=zslab,
                        sums=sums1, sqs=sqs1, invct=invct, epsT=epsT,
                        g_t=gvec[0], b_t=gvec[1])
            _emit_conv(nc, tc, pools, cfg1)
            scl1, nb1 = _emit_norm_epilogue(nc, tc, pools, cfg1)

            ALU = mybir.AluOpType
            for t in range(T):
                yT = small.tile([C, TP], F32, tag="yT")
                nc.vector.tensor_scalar(
                    out=yT[:], in0=zslab[:, t * TP:(t + 1) * TP],
                    scalar1=scl1[:], scalar2=nb1[:],
                    op0=ALU.mult, op1=ALU.add)
                tmp = small.tile([C, TP], F32, tag="yt2")
                nc.scalar.mul(tmp[:], yT[:], SLOPE)
                nc.vector.tensor_tensor(out=yT[:], in0=yT[:], in1=tmp[:],
                                        op=ALU.max)
                ytp = psumY.tile([TP, C], F32, tag="ytp")
                nc.tensor.transpose(ytp[:], yT[:], ident[:C, :C])
                yrow = small.tile([TP, C], F32, tag="yrow")
                nc.scalar.copy(out=yrow[:], in_=ytp[:])
                nc.sync.dma_start(out=y1loc.ap()[t * TP:(t + 1) * TP],
                                  in_=yrow[:])

            # exchange y1
            nc.gpsimd.collective_compute(
                "AllGather", mybir.AluOpType.bypass,
                replica_groups=[list(range(B))],
                ins=[y1loc.ap()],
                outs=[xsrc2.ap()[0:B * RPAD]],
            )
            nc.sync.dma_start(out=xsrc2.ap()[Z2:Z2 + 1], in_=zrow[:])

            cfg2 = dict(midx=m2s, aidx=a2s, xsrc=xsrc2, wslab=w2s,
                        ident=ident, nsrc=nsrc2, auxbase=AB2,
                        sched=(nch2, npass2, ncols2), zslab=zslab,
                        sums=sums2, sqs=sqs2, invct=invct, epsT=epsT,
                        g_t=gvec[2], b_t=gvec[3])
            _emit_conv(nc, tc, pools, cfg2)
            scl2, nb2 = _emit_norm_epilogue(nc, tc, pools, cfg2)

            for t in range(T):
                nT = small.tile([C, TP], F32, tag="nT")
                nc.vector.tensor_scalar(
                    out=nT[:], in0=zslab[:, t * TP:(t + 1) * TP],
                    scalar1=scl2[:], scalar2=nb2[:],
                    op0=ALU.mult, op1=ALU.add)
                ntp = psumY.tile([TP, C], F32, tag="ytp")
                nc.tensor.transpose(ntp[:], nT[:], ident[:C, :C])
                ft = small.tile([TP, C], F32, tag="ft")
                nc.sync.dma_start(out=ft[:],
                                  in_=fpad_t.ap()[t * TP:(t + 1) * TP])
                radd = small.tile([TP, C], F32, tag="radd")
                nc.vector.tensor_add(out=radd[:], in0=ntp[:], in1=ft[:])
                tmp2 = small.tile([TP, C], F32, tag="tmp2")
                nc.scalar.mul(tmp2[:], radd[:], SLOPE)
                orow = small.tile([TP, C], F32, tag="orow")
                nc.vector.tensor_tensor(out=orow[:], in0=radd[:], in1=tmp2[:],
                                        op=ALU.max)
                nc.sync.dma_start(out=out_t.ap()[t * TP:(t + 1) * TP],
                                  in_=orow[:])

    nc.compile()

    in_maps = []
    for c in range(B):
        gb = np.stack([np.asarray(g1, np.float32), np.asarray(b1, np.float32),
                       np.asarray(g2, np.float32), np.asarray(b2, np.float32)])
        in_maps.append({
            "feats": feats,
            "fpad": fpads[c],
            "w1cat": w1cat,
            "w2cat": w2cat,
            "gb": gb,
            "invc": np.array([[1.0 / max(int(counts[c]), 1)]], np.float32),
            "m1": m1[c], "m2": m2[c],
            "a1": a1[c] if acalls1 else np.zeros((TP, 1), np.int32),
            "a2": a2[c] if acalls2 else np.zeros((TP, 1), np.int32),
        })

    if TRACE:
        _register_ntff_hook()
    br = run_bass_kernel_spmd(nc, in_maps, list(range(B)), trace=TRACE)
    LAST_EXEC_NS = br.exec_time_ns

    out = np.empty((N, C), np.float32)
    for c in range(B):
        out[b0[c]:b0[c + 1]] = br.results[c]["out"][:counts[c]]
    return out


# revision 8
# speedup vs baseline: 1.0061x; 1.0001x over previous
"""TRN2 Bass kernel for nn_BasicBlockInstanceNorm (gnn_message_passing).

Algorithm (per conv, twice):
    out[r] = sum_k ( sum_{edges (k,j): out_map[k,j]=r} x[in_map[k,j]] ) @ W[k]
    (aggregate-then-transform: per output row r and offset k, sum the source
    rows first -- the sum commutes with the per-k linear map)
then instance-norm (+residual on conv2) + leaky relu.

Sharding: 8 cores, one batch instance per core (batch_idx is sorted), so
instance-norm stats are core-local.  Each core owns its output rows and
gathers source rows from a replicated source table in its own HBM via
indirect DMA (128 rows / instruction -- the Pool/Q7 descriptor-generation
rate is the kernel bottleneck).  Slots (r, k) with multiple edges are
pre-summed into "aux rows" on device (multi-pass gather-with-accumulate
into compact SBUF blocks, then written to the source table) so the main
pass is exactly one gather per slot.  y1 is exchanged between convs with
an AllGather.  PE does S^T transposes + W_cat matmuls; DVE/ACT drain PSUM
and apply norm/leaky -- all hidden under the gather stream.
"""
import sys
import types

import numpy as np

sys.path.insert(0, "/opt/trn_rl_repo")

import concourse.bacc as bacc
import concourse.bass as bass
import concourse.mybir as mybir
import concourse.tile as tile
from concourse.bass_utils import run_bass_kernel_spmd
from concourse.masks import make_identity

N = 100000
C = 64
K = 27
B = 8
EPS = 1e-5
SLOPE = 0.01
TP = 128
T = 100            # output tiles per core (recomputed per inputs)
RPAD = T * TP      # padded rows per core (recomputed per inputs)
ACOLS = 64         # aux slots per chunk = 128*ACOLS
AUX_CHUNK = TP * ACOLS
DUMMY = np.int32(1 << 30)

TRACE = False
LAST_EXEC_NS = None

F32 = mybir.dt.float32
I32 = mybir.dt.int32


def _register_ntff_hook():
    import antenv
    if "antenv.axon_hooks" not in sys.modules:
        mod = types.ModuleType("antenv.axon_hooks")
        mod._hook = None
        def set_hook(h):
            mod._hook = h
        def get_hook():
            return mod._hook
        mod.set_axon_ntff_profile_hook = set_hook
        mod.get_axon_ntff_profile_hook = get_hook
        sys.modules["antenv.axon_hooks"] = mod
        antenv.axon_hooks = mod
    m = sys.modules["antenv.axon_hooks"]
    if m.get_axon_ntff_profile_hook() is None:
        from trn_agent_boot.trn_boot import _ntff_profile_via_ctypes
        m.set_axon_ntff_profile_hook(
            _ntff_profile_via_ctypes("/opt/axon/libaxon_pjrt.so")
        )


def _build_edge_structure(in_map, out_map, b0):
    """Per-core slot structure shared by both convs.

    Returns per-core dict with:
      single_slot, single_src   -- slots with exactly one edge (global src ids)
      aux_counts [A]            -- per aux slot edge count (desc-sorted)
      aux_slot [A]              -- slot id of each aux slot
      aux_srcs  [A, maxc]       -- global src ids per aux slot (padded -1)
    slot id = r*K + k  (r = core-local row).
    """
    src = in_map.ravel().astype(np.int64)          # edge order: k-major
    dst = out_map.ravel().astype(np.int64)
    k_arr = np.repeat(np.arange(K, dtype=np.int64), N)
    core = np.searchsorted(b0[1:-1], dst, side="right")
    r = dst - b0[core]
    gslot = (core * RPAD + r) * K + k_arr          # globally unique slot id
    order = np.argsort(gslot, kind="stable")
    gs = gslot[order]
    ss = src[order]
    uniq, start, cnt = np.unique(gs, return_index=True, return_counts=True)
    out = []
    for c in range(B):
        lo = np.searchsorted(uniq, c * RPAD * K)
        hi = np.searchsorted(uniq, (c + 1) * RPAD * K)
        u = uniq[lo:hi] - c * RPAD * K
        st = start[lo:hi]
        ct = cnt[lo:hi]
        singles = ct == 1
        single_slot = u[singles]
        single_src = ss[st[singles]]
        dmask = ct >= 2
        d_slot = u[dmask]
        d_start = st[dmask]
        d_cnt = ct[dmask]
        ordd = np.argsort(-d_cnt, kind="stable")
        d_slot = d_slot[ordd]
        d_start = d_start[ordd]
        d_cnt = d_cnt[ordd]
        maxc = int(d_cnt[0]) if len(d_cnt) else 0
        srcs = np.full((len(d_slot), maxc), -1, np.int64)
        for p in range(maxc):
            m = d_cnt > p
            srcs[m, p] = ss[d_start[m] + p]
        out.append(dict(single_slot=single_slot, single_src=single_src,
                        aux_cnt=d_cnt, aux_slot=d_slot, aux_srcs=srcs))
    return out


def _build_conv_arrays(structs, enc, zrow, auxbase):
    """Build per-core main idx [128, T*K] and aux pass arrays + global schedule.

    enc: vectorized map global-src -> xsrc row.  Returns (m_list, a_list,
    nchunks, npasses[chunk], ncols[chunk][pass], acalls).
    """
    maxA = max(len(s["aux_slot"]) for s in structs)
    nchunks = max(1, -(-maxA // AUX_CHUNK))
    apad = nchunks * AUX_CHUNK
    # global schedule
    npasses = []
    ncols = []
    for q in range(nchunks):
        pmax = 1
        for s in structs:
            cnts = s["aux_cnt"][q * AUX_CHUNK:(q + 1) * AUX_CHUNK]
            if len(cnts):
                pmax = max(pmax, int(cnts[0]))
        npasses.append(pmax)
        cols_q = []
        for p in range(pmax):
            if p == 0:
                cols_q.append(ACOLS)
                continue
            w = 1
            for s in structs:
                cnts = s["aux_cnt"][q * AUX_CHUNK:(q + 1) * AUX_CHUNK]
                pref = int((cnts > p).sum())
                w = max(w, -(-pref // TP)) if pref else w
            cols_q.append(w)
        ncols.append(cols_q)
    acalls = sum(sum(cq) for cq in ncols)

    m_list, a_list = [], []
    for s in structs:
        m = np.full((TP, T * K), zrow, np.int32)
        sl = s["single_slot"]
        rr = sl // K
        kk = sl % K
        tt = rr // TP
        pp = rr % TP
        m[pp, tt * K + kk] = enc(s["single_src"])
        # aux slots: main idx points at the aux row
        A = len(s["aux_slot"])
        a_ids = np.arange(apad, dtype=np.int64)
        chunk = a_ids // AUX_CHUNK
        w = a_ids % AUX_CHUNK
        dram_row = chunk * AUX_CHUNK + (w % TP) * ACOLS + (w // TP)
        if A:
            sl = s["aux_slot"]
            rr = sl // K
            kk = sl % K
            tt = rr // TP
            pp = rr % TP
            m[pp, tt * K + kk] = (auxbase + dram_row[:A]).astype(np.int32)
        a = np.full((TP, acalls), DUMMY, np.int32)
        col = 0
        for q in range(nchunks):
            base = q * AUX_CHUNK
            for p in range(npasses[q]):
                for cq in range(ncols[q][p]):
                    ws = base + cq * TP + np.arange(TP)
                    vals = np.full(TP, DUMMY, np.int32)
                    if p == 0:
                        vals[:] = zrow
                    inr = ws < A
                    if inr.any():
                        wi = ws[inr]
                        have = s["aux_srcs"].shape[1] > p
                        if have:
                            sv = s["aux_srcs"][wi, p]
                            ok = sv >= 0
                            tmp = vals[inr]
                            tmp[ok] = enc(sv[ok])
                            vals[inr] = tmp
                    a[:, col] = vals
                    col += 1
        m_list.append(m)
        a_list.append(a)
    return m_list, a_list, nchunks, npasses, ncols, acalls, apad


def _emit_conv(nc, tc, pools, cfg):
    """Emit one conv's instructions."""
    (const_p, s_pool, st_pool, aux_pool, small, psumT, psumZ, psumY) = pools
    midx = cfg["midx"]
    aidx = cfg["aidx"]
    xsrc = cfg["xsrc"]
    wslab = cfg["wslab"]
    ident = cfg["ident"]
    nsrc = cfg["nsrc"]
    auxbase = cfg["auxbase"]
    nchunks, npasses, ncols = cfg["sched"]
    zslab = cfg["zslab"]
    sums = cfg["sums"]
    sqs = cfg["sqs"]

    AX = mybir.AxisListType.X
    # ---- aux build ----
    acall = 0
    for q in range(nchunks):
        slab = aux_pool.tile([TP, ACOLS * C], F32, tag="aux")
        for p in range(npasses[q]):
            for cq in range(ncols[q][p]):
                dst = slab[:, cq * C:(cq + 1) * C] if p > 0 else None
                col = aidx[:, acall:acall + 1]
                acall += 1
                if p == 0:
                    nc.gpsimd.indirect_dma_start(
                        out=slab[:, cq * C:(cq + 1) * C], out_offset=None,
                        in_=xsrc.ap(),
                        in_offset=bass.IndirectOffsetOnAxis(ap=col, axis=0))
                else:
                    nc.gpsimd.indirect_dma_start(
                        out=dst, out_offset=None, in_=xsrc.ap(),
                        in_offset=bass.IndirectOffsetOnAxis(ap=col, axis=0),
                        bounds_check=nsrc - 1, oob_is_err=False,
                        compute_op=mybir.AluOpType.add)
        row0 = auxbase + q * AUX_CHUNK
        nc.sync.dma_start(
            out=xsrc.ap()[row0:row0 + AUX_CHUNK].rearrange(
                "(p a) c -> p (a c)", p=TP),
            in_=slab[:])

    # ---- main pass + matmul + stats ----
    for t in range(T):
        S = s_pool.tile([TP, 14 * TP], F32, tag="S")
        nc.vector.memset(S[:, K * C:], 0.0)
        for k in range(K):
            nc.gpsimd.indirect_dma_start(
                out=S[:, k * C:(k + 1) * C], out_offset=None, in_=xsrc.ap(),
                in_offset=bass.IndirectOffsetOnAxis(
                    ap=midx[:, t * K + k:t * K + k + 1], axis=0))
        zT = psumZ.tile([C, TP], F32, tag="zT")
        for i in range(14):
            tp_ = psumT.tile([TP, TP], F32, tag="tp")
            nc.tensor.transpose(tp_[:], S[:, i * TP:(i + 1) * TP], ident[:])
            stc = st_pool.tile([TP, TP], F32, tag="st")
            if i % 2 == 0:
                nc.vector.tensor_copy(out=stc[:], in_=tp_[:])
            else:
                nc.scalar.copy(out=stc[:], in_=tp_[:])
            nc.tensor.matmul(zT[:], lhsT=wslab[:, i * C:(i + 1) * C],
                             rhs=stc[:], start=(i == 0), stop=(i == 13))
        nc.vector.reduce_sum(out=sums[:, t:t + 1], in_=zT[:], axis=AX)
        sq = small.tile([C, TP], F32, tag="sq")
        nc.scalar.activation(out=sq[:], in_=zT[:],
                             func=mybir.ActivationFunctionType.Square,
                             accum_out=sqs[:, t:t + 1])
        nc.vector.tensor_copy(out=zslab[:, t * TP:(t + 1) * TP], in_=zT[:])


def _emit_norm_epilogue(nc, tc, pools, cfg):
    (const_p, s_pool, st_pool, aux_pool, small, psumT, psumZ, psumY) = pools
    zslab = cfg["zslab"]
    sums = cfg["sums"]
    sqs = cfg["sqs"]
    invct = cfg["invct"]
    epsT = cfg["epsT"]
    g_t = cfg["g_t"]
    b_t = cfg["b_t"]
    ident = cfg["ident"]
    AX = mybir.AxisListType.X
    ALU = mybir.AluOpType

    sumv = small.tile([C, 1], F32, tag="n1")
    nc.vector.reduce_sum(out=sumv[:], in_=sums[:], axis=AX)
    sqv = small.tile([C, 1], F32, tag="n2")
    nc.vector.reduce_sum(out=sqv[:], in_=sqs[:], axis=AX)
    mean = small.tile([C, 1], F32, tag="n3")
    nc.vector.tensor_mul(out=mean[:], in0=sumv[:], in1=invct[:])
    msq = small.tile([C, 1], F32, tag="n4")
    nc.vector.tensor_mul(out=msq[:], in0=sqv[:], in1=invct[:])
    m2t = small.tile([C, 1], F32, tag="n5")
    nc.vector.tensor_mul(out=m2t[:], in0=mean[:], in1=mean[:])
    var = small.tile([C, 1], F32, tag="n6")
    nc.vector.tensor_tensor(out=var[:], in0=msq[:], in1=m2t[:],
                            op=ALU.subtract)
    stdt = small.tile([C, 1], F32, tag="n7")
    nc.scalar.activation(out=stdt[:], in_=var[:],
                         func=mybir.ActivationFunctionType.Sqrt,
                         bias=epsT[:], scale=1.0)
    rstd = small.tile([C, 1], F32, tag="n8")
    nc.vector.reciprocal(out=rstd[:], in_=stdt[:])
    scl = small.tile([C, 1], F32, tag="n9")
    nc.vector.tensor_mul(out=scl[:], in0=g_t[:], in1=rstd[:])
    tmpb = small.tile([C, 1], F32, tag="n10")
    nc.vector.tensor_mul(out=tmpb[:], in0=mean[:], in1=scl[:])
    nbias = small.tile([C, 1], F32, tag="n11")
    nc.vector.tensor_tensor(out=nbias[:], in0=b_t[:], in1=tmpb[:],
                            op=ALU.subtract)
    return scl, nbias


def _numpy_fallback(feats, W1, g1, b1, W2, g2, b2, in_map, out_map, batch_idx):
    x = feats.astype(np.float64)
    def conv(v, W):
        out = np.zeros_like(v)
        for k in range(K):
            np.add.at(out, out_map[k], v[in_map[k]] @ W[k])
        return out
    def inorm(v, g, b):
        o = np.empty_like(v)
        for c in range(B):
            m = batch_idx == c
            if not m.any():
                continue
            mu = v[m].mean(0)
            va = v[m].var(0)
            o[m] = (v[m] - mu) / np.sqrt(va + EPS) * g + b
        return o
    lr = lambda v: np.where(v > 0, v, SLOPE * v)
    o = lr(inorm(conv(x, W1), g1, b1))
    o = inorm(conv(o, W2), g2, b2) + x
    return lr(o).astype(np.float32)


def kernel(feats, W1, g1, b1, W2, g2, b2, in_map, out_map, batch_idx):
    global LAST_EXEC_NS
    feats = np.ascontiguousarray(np.asarray(feats, np.float32))
    in_map = np.asarray(in_map, np.int32)
    out_map = np.asarray(out_map, np.int32)
    batch_idx = np.asarray(batch_idx, np.int32)

    global T, RPAD
    counts = np.bincount(batch_idx, minlength=B)
    T = max(1, -(-int(counts.max()) // TP))
    RPAD = T * TP
    b0 = np.zeros(B + 1, np.int64)
    b0[1:] = np.cumsum(counts)
    assert b0[-1] == N
    if T > 110:
        # Pathologically skewed batch sizes would overflow the SBUF slabs
        # sized for the spec's uniform sorted_randint fill; fall back to a
        # host implementation rather than crash.
        return _numpy_fallback(feats, W1, g1, b1, W2, g2, b2, in_map,
                               out_map, batch_idx)

    structs = _build_edge_structure(in_map, out_map, b0)

    # conv1 source table: [feats | zero | aux1]
    Z1 = N
    AB1 = N + 1
    m1, a1, nch1, npass1, ncols1, acalls1, apad1 = _build_conv_arrays(
        structs, lambda s: np.asarray(s, np.int64).astype(np.int32), Z1, AB1)
    # conv2 source table: [y1 padded blocks | zero | aux2]
    Z2 = B * RPAD
    AB2 = B * RPAD + 1
    src_core = np.searchsorted(b0[1:-1], np.arange(N), side="right")
    enc2_tab = (src_core * RPAD + (np.arange(N) - b0[src_core])).astype(np.int32)
    m2, a2, nch2, npass2, ncols2, acalls2, apad2 = _build_conv_arrays(
        structs, lambda s: enc2_tab[np.asarray(s, np.int64)], Z2, AB2)

    w1cat = np.zeros((14 * TP, C), np.float32)
    w1cat[:K * C] = np.asarray(W1, np.float32).reshape(K * C, C)
    w2cat = np.zeros((14 * TP, C), np.float32)
    w2cat[:K * C] = np.asarray(W2, np.float32).reshape(K * C, C)

    fpads = []
    for c in range(B):
        fp = np.zeros((RPAD, C), np.float32)
        fp[:counts[c]] = feats[b0[c]:b0[c + 1]]
        fpads.append(fp)

    nsrc1 = N + 1 + apad1
    nsrc2 = B * RPAD + 1 + apad2

    # ---------------- build program ----------------
    nc = bacc.Bacc("TRN2", target_bir_lowering=False, debug=False,
                   num_devices=B)
    feats_t = nc.dram_tensor("feats", [N, C], F32, kind="ExternalInput")
    fpad_t = nc.dram_tensor("fpad", [RPAD, C], F32, kind="ExternalInput")
    w1_t = nc.dram_tensor("w1cat", [14 * TP, C], F32, kind="ExternalInput")
    w2_t = nc.dram_tensor("w2cat", [14 * TP, C], F32, kind="ExternalInput")
    gb_t = nc.dram_tensor("gb", [4, C], F32, kind="ExternalInput")
    inv_t = nc.dram_tensor("invc", [1, 1], F32, kind="ExternalInput")
    m1_t = nc.dram_tensor("m1", [TP, T * K], I32, kind="ExternalInput")
    m2_t = nc.dram_tensor("m2", [TP, T * K], I32, kind="ExternalInput")
    a1_t = nc.dram_tensor("a1", [TP, max(acalls1, 1)], I32, kind="ExternalInput")
    a2_t = nc.dram_tensor("a2", [TP, max(acalls2, 1)], I32, kind="ExternalInput")
    out_t = nc.dram_tensor("out", [RPAD, C], F32, kind="ExternalOutput")
    xsrc1 = nc.dram_tensor("xsrc1", [nsrc1, C], F32)
    xsrc2 = nc.dram_tensor("xsrc2", [nsrc2, C], F32)
    y1loc = nc.dram_tensor("y1loc", [RPAD, C], F32)

    with tile.TileContext(nc) as tc:
        with (
            tc.tile_pool(name="const", bufs=1) as const_p,
            tc.tile_pool(name="S", bufs=5) as s_pool,
            tc.tile_pool(name="st", bufs=6) as st_pool,
            tc.tile_pool(name="aux", bufs=3) as aux_pool,
            tc.tile_pool(name="small", bufs=4) as small,
            tc.tile_pool(name="zsl", bufs=1) as zpool,
            tc.tile_pool(name="psT", bufs=3, space="PSUM") as psumT,
            tc.tile_pool(name="psZ", bufs=2, space="PSUM") as psumZ,
            tc.tile_pool(name="psY", bufs=2, space="PSUM") as psumY,
        ):
            pools = (const_p, s_pool, st_pool, aux_pool, small, psumT,
                     psumZ, psumY)
            ident = const_p.tile([TP, TP], F32)
            make_identity(nc, ident[:])
            w1s = const_p.tile([TP, 14 * C], F32)
            nc.sync.dma_start(
                out=w1s[:].rearrange("p (i c) -> p i c", c=C),
                in_=w1_t.ap().rearrange("(i p) c -> p i c", p=TP))
            w2s = const_p.tile([TP, 14 * C], F32)
            nc.sync.dma_start(
                out=w2s[:].rearrange("p (i c) -> p i c", c=C),
                in_=w2_t.ap().rearrange("(i p) c -> p i c", p=TP))
            gvec = []
            for i in range(4):
                gt = const_p.tile([C, 1], F32, tag=f"gb{i}")
                nc.sync.dma_start(out=gt[:], in_=gb_t.ap()[i:i+1].rearrange("a c -> c a"))
                gvec.append(gt)
            invct = const_p.tile([C, 1], F32)
            nc.sync.dma_start(
                out=invct[:],
                in_=bass.AP(tensor=inv_t, offset=0, ap=[[0, C], [1, 1]]))
            epsT = const_p.tile([C, 1], F32)
            nc.vector.memset(epsT[:], EPS)
            zrow = const_p.tile([1, C], F32)
            nc.vector.memset(zrow[:], 0.0)

            m1s = const_p.tile([TP, T * K], I32)
            nc.sync.dma_start(out=m1s[:], in_=m1_t.ap())
            m2s = const_p.tile([TP, T * K], I32)
            nc.sync.dma_start(out=m2s[:], in_=m2_t.ap())
            a1s = const_p.tile([TP, max(acalls1, 1)], I32)
            nc.sync.dma_start(out=a1s[:], in_=a1_t.ap())
            a2s = const_p.tile([TP, max(acalls2, 1)], I32)
            nc.sync.dma_start(out=a2s[:], in_=a2_t.ap())

            zslab = zpool.tile([C, RPAD], F32)
            sums1 = const_p.tile([C, T], F32, tag="sums1")
            sqs1 = const_p.tile([C, T], F32, tag="sqs1")
            sums2 = const_p.tile([C, T], F32, tag="sums2")
            sqs2 = const_p.tile([C, T], F32, tag="sqs2")

            # init conv1 source table
            nc.sync.dma_start(out=xsrc1.ap()[0:N], in_=feats_t.ap())
            nc.sync.dma_start(out=xsrc1.ap()[N:N + 1], in_=zrow[:])

            cfg1 = dict(midx=m1s, aidx=a1s, xsrc=xsrc1, wslab=w1s,
                        ident=ident, nsrc=nsrc1, auxbase=AB1,
                        sched=(nch1, npass1, ncols1), zslab=zslab,
                        sums=sums1, sqs=sqs1, invct=invct, epsT=epsT,
                        g_t=gvec[0], b_t=gvec[1])
            _emit_conv(nc, tc, pools, cfg1)
            scl1, nb1 = _emit_norm_epilogue(nc, tc, pools, cfg1)

            ALU = mybir.AluOpType
            for t in range(T):
                yT = small.tile([C, TP], F32, tag="yT")
                nc.vector.tensor_scalar(
                    out=yT[:], in0=zslab[:, t * TP:(t + 1) * TP],
                    scalar1=scl1[:], scalar2=nb1[:],
                    op0=ALU.mult, op1=ALU.add)
                tmp = small.tile([C, TP], F32, tag="yt2")
                nc.scalar.mul(tmp[:], yT[:], SLOPE)
                nc.vector.tensor_tensor(out=yT[:], in0=yT[:], in1=tmp[:],
                                        op=ALU.max)
                ytp = psumY.tile([TP, C], F32, tag="ytp")
                nc.tensor.transpose(ytp[:], yT[:], ident[:C, :C])
                yrow = small.tile([TP, C], F32, tag="yrow")
                nc.scalar.copy(out=yrow[:], in_=ytp[:])
                nc.sync.dma_start(out=y1loc.ap()[t * TP:(t + 1) * TP],
                                  in_=yrow[:])

            # exchange y1
            nc.gpsimd.collective_compute(
                "AllGather", mybir.AluOpType.bypass,
                replica_groups=[list(range(B))],
                ins=[y1loc.ap()],
                outs=[xsrc2.ap()[0:B * RPAD]],
            )
            nc.sync.dma_start(out=xsrc2.ap()[Z2:Z2 + 1], in_=zrow[:])

            cfg2 = dict(midx=m2s, aidx=a2s, xsrc=xsrc2, wslab=w2s,
                        ident=ident, nsrc=nsrc2, auxbase=AB2,
                        sched=(nch2, npass2, ncols2), zslab=zslab,
                        sums=sums2, sqs=sqs2, invct=invct, epsT=epsT,
                        g_t=gvec[2], b_t=gvec[3])
            _emit_conv(nc, tc, pools, cfg2)
            scl2, nb2 = _emit_norm_epilogue(nc, tc, pools, cfg2)

            for t in range(T):
                nT = small.tile([C, TP], F32, tag="nT")
                nc.vector.tensor_scalar(
                    out=nT[:], in0=zslab[:, t * TP:(t + 1) * TP],
                    scalar1=scl2[:], scalar2=nb2[:],
                    op0=ALU.mult, op1=ALU.add)
                ntp = psumY.tile([TP, C], F32, tag="ytp")
                nc.tensor.transpose(ntp[:], nT[:], ident[:C, :C])
                ft = small.tile([TP, C], F32, tag="ft")
                nc.sync.dma_start(out=ft[:],
                                  in_=fpad_t.ap()[t * TP:(t + 1) * TP])
                radd = small.tile([TP, C], F32, tag="radd")
                nc.vector.tensor_add(out=radd[:], in0=ntp[:], in1=ft[:])
                tmp2 = small.tile([TP, C], F32, tag="tmp2")
                nc.scalar.mul(tmp2[:], radd[:], SLOPE)
                orow = small.tile([TP, C], F32, tag="orow")
                nc.vector.tensor_tensor(out=orow[:], in0=radd[:], in1=tmp2[:],
                                        op=ALU.max)
                nc.sync.dma_start(out=out_t.ap()[t * TP:(t + 1) * TP],
                                  in_=orow[:])

    nc.compile()

    in_maps = []
    for c in range(B):
        gb = np.stack([np.asarray(g1, np.float32), np.asarray(b1, np.float32),
                       np.asarray(g2, np.float32), np.asarray(b2, np.float32)])
        in_maps.append({
            "feats": feats,
            "fpad": fpads[c],
            "w1cat": w1cat,
            "w2cat": w2cat,
            "gb": gb,
            "invc": np.array([[1.0 / max(int(counts[c]), 1)]], np.float32),
            "m1": m1[c], "m2": m2[c],
            "a1": a1[c] if acalls1 else np.zeros((TP, 1), np.int32),
            "a2": a2[c] if acalls2 else np.zeros((TP, 1), np.int32),
        })

    if TRACE:
        _register_ntff_hook()
    br = run_bass_kernel_spmd(nc, in_maps, list(range(B)), trace=TRACE)
    LAST_EXEC_NS = br.exec_time_ns

    out = np.empty((N, C), np.float32)
    for c in range(B):
        out[b0[c]:b0[c + 1]] = br.results[c]["out"][:counts[c]]
    return out
